# revision 10
# baseline (speedup 1.0000x reference)
"""GAT message-passing network on 8 Trainium2 NeuronCores.

Strategy: nodes (padded to 20480) are sorted by in-degree and dealt into
160 tiles of 128; core c owns tiles i*8+c (20 iterations). Per-edge work
is laid out ELL-style: each node-tile gathers its (padded) in-neighbor
rows [xl+b | al_s | al_d] with one bulk dma_gather, so segment softmax
and the weighted aggregation become dense row ops. Self loops are slot 0
with cls := cnt1/deg (makes the generic 2-class attention term equal the
'mean' fill value). Layers communicate via AllGather of each core's
2560-row shard. The edge-scoring MLP runs on deduped (src>dst, sorted)
pairs; the per-graph softmax uses a gather-ELL over graphs + AllReduce.
"""
import os
import numpy as np
from contextlib import ExitStack

N_NODES = 20000
NPAD = 20480
NC = 8
TILES = NPAD // 128          # 160
ITERS = TILES // NC          # 20
NUM_GRAPHS = 200
EMB = 16
GCN = [32, 64, 128]
CI = [8, 32, 64]             # padded in-channels per layer
ROWL = [64, 128, 192]        # gathered row length per layer (co+2 padded to 64x)
PAIR_CHUNK = 512
F32 = np.float32


def _pack_idx(flat):
    """dma_gather index packing: flat[i] consumed as idxs[i%16, i//16], tiled x8."""
    flat = np.asarray(flat, np.int64)
    assert flat.size % 16 == 0 and flat.max() < 32768 and flat.min() >= 0
    w = flat.reshape(-1, 16).T.astype(np.int16)
    return np.tile(w, (8, 1))


def _preprocess(x, edges, edge_attr, detector_labels, batch_labels):
    """Pure index/layout preprocessing. Returns per-core input dicts + host outputs."""
    E = edges.shape[1]
    src = edges[0].astype(np.int64)
    dst = edges[1].astype(np.int64)
    cls = edge_attr[:, 1].astype(F32)

    deg = np.bincount(dst, minlength=NPAD).astype(np.int64)
    order = np.argsort(deg, kind="stable")          # ascending degree, dummies first
    pos = np.empty(NPAD, np.int64)
    pos[order] = np.arange(NPAD)

    # AG row of a global node-slot: slot = t*128+q, tile t -> core t%8 row
    slot = pos                                       # node -> global slot
    t_of = slot // 128
    agrow_of_node = (t_of % NC) * (ITERS * 128) + (t_of // NC) * 128 + (slot % 128)

    # per-iteration slot count (same across cores)
    S_list = (deg[order].reshape(ITERS, NC * 128).max(1) + 1).astype(np.int64)

    # CSR of in-edges by dst
    e_order = np.argsort(dst, kind="stable")
    dst_s = dst[e_order]
    src_s = src[e_order]
    cls_s = cls[e_order]
    starts = np.searchsorted(dst_s, np.arange(NPAD))
    cnt1 = np.bincount(dst, weights=(cls > 0.5), minlength=NPAD)
    f1 = cnt1 / np.maximum(deg, 1)

    core_in = [{} for _ in range(NC)]
    for c in range(NC):
        idx_cols, cls_cols, mask_cols = [], [], []
        for i in range(ITERS):
            S = int(S_list[i])
            nodes = order[(i * NC + c) * 128:(i * NC + c) * 128 + 128]
            d = deg[nodes]
            k = np.arange(1, S)[None, :]            # edge slots 1..S-1
            valid = k <= d[:, None]
            epos = starts[nodes][:, None] + (k - 1)
            epos = np.where(valid, epos, 0)
            ell_i = np.zeros((128, S), np.int64)
            ell_i[:, 0] = agrow_of_node[nodes]      # self slot
            ell_i[:, 1:] = np.where(valid, agrow_of_node[src_s[epos]], 0)
            ell_c = np.zeros((128, S), F32)
            ell_c[:, 0] = f1[nodes]
            ell_c[:, 1:] = np.where(valid, cls_s[epos], 0.0)
            ell_m = np.zeros((128, S), F32)
            ell_m[:, 0] = 1.0
            ell_m[:, 1:] = valid
            idx_cols.append(_pack_idx(ell_i.T.ravel()))   # flat[k*128+p]
            cls_cols.append(ell_c)
            mask_cols.append(ell_m)
        core_in[c]["ell_idx"] = np.concatenate(idx_cols, 1)
        core_in[c]["ell_cls"] = np.concatenate(cls_cols, 1)
        core_in[c]["ell_mask"] = np.concatenate(mask_cols, 1)
        # x rows for my tiles, padded to 8 cols
        nodes_c = order[(np.arange(ITERS)[:, None] * NC + c) * 128
                        + np.arange(128)[None, :]].ravel()
        xm = np.zeros((ITERS * 128, 8), F32)
        real = nodes_c < N_NODES
        xm[real, :x.shape[1]] = x[nodes_c[real]]
        core_in[c]["x_mine"] = xm

    # ---- SplitSyndromes (host, index-only) ----
    valid_e = detector_labels[edges[0]] & detector_labels[edges[1]]
    keep = valid_e & (src > dst)
    e3 = edges[:, keep]
    a3c = cls[keep]
    key = e3[0].astype(np.int64) * detector_labels.shape[0] + e3[1]
    perm = np.argsort(key, kind="stable")
    e3 = e3[:, perm]
    a3c = a3c[perm]
    ES = e3.shape[1]
    assert ES % 2 == 0
    assert (e3[:, ::2] == e3[:, 1::2]).all(), "pair-dedup structure violated"
    NP2 = ES // 2
    a_n = e3[0, ::2].astype(np.int64)
    b_n = e3[1, ::2].astype(np.int64)
    ec_host = a3c[::2].astype(F32)                  # class of first-of-pair (argmin=0 on ties)
    e_out = e3[:, ::2].astype(np.int32)
    eb = batch_labels[a_n].astype(np.int64)

    ppc = NP2 // NC                                  # 25000
    PPC = ((ppc + PAIR_CHUNK - 1) // PAIR_CHUNK) * PAIR_CHUNK
    NCH = PPC // PAIR_CHUNK
    NCOL = PPC // 128
    KG = 0
    for c in range(NC):
        KG = max(KG, np.bincount(eb[c * ppc:(c + 1) * ppc], minlength=NUM_GRAPHS).max())
    KG = int(((KG + 31) // 32) * 32)

    for c in range(NC):
        ua = np.zeros(PPC, np.int64)
        vb = np.zeros(PPC, np.int64)
        ebp = np.zeros(PPC, np.int64)
        msk = np.zeros(PPC, F32)
        sl = slice(c * ppc, (c + 1) * ppc)
        ua[:ppc] = agrow_of_node[a_n[sl]]
        vb[:ppc] = agrow_of_node[b_n[sl]]
        ebp[:ppc] = eb[sl]
        msk[:ppc] = 1.0
        core_in[c]["ua_idx"] = np.concatenate(
            [_pack_idx(ua[j * PAIR_CHUNK:(j + 1) * PAIR_CHUNK]) for j in range(NCH)], 1)
        core_in[c]["vb_idx"] = np.concatenate(
            [_pack_idx(vb[j * PAIR_CHUNK:(j + 1) * PAIR_CHUNK]) for j in range(NCH)], 1)
        core_in[c]["eb_idx"] = _pack_idx(ebp)        # S_table rows, one gather
        core_in[c]["pair_mask"] = msk.reshape(NCOL, 128).T.copy()  # [128, NCOL]
        # graph-ELL: 2 tiles of 128 graphs, KG slots -> local pair index (dummy=ppc)
        gell = np.full((256, KG), ppc, np.int64)     # ppc is a masked dummy pair
        for g in range(NUM_GRAPHS):
            mem = np.nonzero(ebp[:ppc] == g)[0]
            gell[g, :mem.size] = mem
        core_in[c]["gph_idx"] = np.concatenate(
            [_pack_idx(gell[t * 128:(t + 1) * 128].T.ravel()) for t in range(2)], 1)

    meta = dict(S_list=[int(s) for s in S_list], PPC=PPC, NCH=NCH, NCOL=NCOL, KG=KG, ppc=ppc)
    return core_in, meta, e_out, ec_host


def _build_program(meta, params):
    import concourse.bass as bass
    import concourse.bacc as bacc
    import concourse.tile as tile
    import concourse.mybir as mybir
    from concourse.masks import make_identity

    S_list = meta["S_list"]; PPC = meta["PPC"]; NCH = meta["NCH"]
    NCOL = meta["NCOL"]; KG = meta["KG"]
    SSUM = sum(S_list)
    AOp = mybir.AluOpType
    ACT = mybir.ActivationFunctionType
    f32 = mybir.dt.float32
    i16 = mybir.dt.int16
    SHARD = ITERS * 128

    nc = bacc.Bacc("TRN2", target_bir_lowering=False, debug=False, num_devices=NC)

    def din(name, shape, dt=f32):
        return nc.dram_tensor(name, shape, dt, kind="ExternalInput")

    x_mine = din("x_mine", [SHARD, 8])
    ell_idx = din("ell_idx", [128, 8 * SSUM], i16)
    ell_cls = din("ell_cls", [128, SSUM])
    ell_mask = din("ell_mask", [128, SSUM])
    ua_idx = din("ua_idx", [128, NCH * PAIR_CHUNK // 16], i16)
    vb_idx = din("vb_idx", [128, NCH * PAIR_CHUNK // 16], i16)
    eb_idx = din("eb_idx", [128, PPC // 16], i16)
    pair_mask = din("pair_mask", [128, NCOL])
    gph_idx = din("gph_idx", [128, 2 * KG * 8], i16)
    emb_W = din("emb_W", [2, EMB]); emb_b = din("emb_b", [1, EMB])
    Ws, asr, adr, Wer, aer, br = [], [], [], [], [], []
    for l in range(3):
        Ws.append(din(f"W{l}", [CI[l], GCN[l]]))
        asr.append(din(f"as{l}", [1, GCN[l]]))
        adr.append(din(f"ad{l}", [1, GCN[l]]))
        Wer.append(din(f"We{l}", [EMB, GCN[l]]))
        aer.append(din(f"ae{l}", [1, GCN[l]]))
        br.append(din(f"b{l}", [1, GCN[l]]))
    d0wp = din("d0wp", [128, 256])
    d0br = din("d0br", [1, 128])
    d1w = din("d1w", [128, 64]); d1br = din("d1br", [1, 64])
    outw = din("outw", [1, 64]); outb = din("outb", [1, 1])

    sm_out = nc.dram_tensor("sm_out", [PPC], f32, kind="ExternalOutput")

    ag_in = [nc.dram_tensor(f"ag_in{l}", [SHARD, ROWL[l]], f32) for l in range(3)]
    ag_in.append(nc.dram_tensor("ag_in3", [SHARD, 256], f32))
    xlp = [nc.dram_tensor(f"xlp{l}", [NPAD, ROWL[l]], f32, addr_space="Shared")
           for l in range(3)]
    uv = nc.dram_tensor("uv", [NPAD, 256], f32, addr_space="Shared")
    ex_table = nc.dram_tensor("ex_table", [PPC, 64], f32)
    ar_in = nc.dram_tensor("ar_in", [256, 1], f32)
    ar_out = nc.dram_tensor("ar_out", [256, 1], f32, addr_space="Shared")
    s_table = nc.dram_tensor("s_table", [256, 64], f32)

    RG = [list(range(NC))]

    with tile.TileContext(nc) as tc, ExitStack() as ctx:
        const = ctx.enter_context(tc.tile_pool(name="const", bufs=1))
        work = ctx.enter_context(tc.tile_pool(name="work", bufs=2))
        small = ctx.enter_context(tc.tile_pool(name="small", bufs=3))
        big = ctx.enter_context(tc.tile_pool(name="big", bufs=1))
        ps = ctx.enter_context(tc.tile_pool(name="ps", bufs=2, space="PSUM"))
        psb = ctx.enter_context(tc.tile_pool(name="psb", bufs=2, space="PSUM"))

        # ---------- resident index/class/mask data ----------
        ell_idx_t = const.tile([128, 8 * SSUM], i16)
        nc.sync.dma_start(ell_idx_t[:], ell_idx.ap())
        ell_cls_t = const.tile([128, SSUM], f32)
        nc.sync.dma_start(ell_cls_t[:], ell_cls.ap())
        ell_mask_t = const.tile([128, SSUM], f32)
        nc.sync.dma_start(ell_mask_t[:], ell_mask.ap())
        pm_t = const.tile([128, NCOL], f32)
        nc.sync.dma_start(pm_t[:], pair_mask.ap())

        ident = const.tile([128, 128], f32)
        make_identity(nc, ident[:])
        ones_col = const.tile([1, 128], f32)
        nc.gpsimd.memset(ones_col[:], 1.0)

        def bcast_row(row_ap, n):
            """[1, n] SBUF/DRAM row -> [128, n] SBUF tile."""
            r = small.tile([1, n], f32)
            nc.sync.dma_start(r[:], row_ap)
            p = psb.tile([128, 256], f32, tag="pb")
            nc.tensor.matmul(out=p[:, :n], lhsT=ones_col[:], rhs=r[:], start=True, stop=True)
            o = const.tile([128, n], f32, tag=f"bc{n}_{bcast_row.n}")
            bcast_row.n += 1
            nc.vector.tensor_copy(o[:], p[:, :n])
            return o
        bcast_row.n = 0

        # ---------- per-layer parameter staging ----------
        # c_table = tanh(emb_W + emb_b)  [2, 16]
        embw_t = small.tile([2, EMB], f32)
        nc.sync.dma_start(embw_t[:], emb_W.ap())
        embb_r = small.tile([1, EMB], f32)
        nc.sync.dma_start(embb_r[:], emb_b.ap())
        one2 = small.tile([1, 2], f32)
        nc.gpsimd.memset(one2[:], 1.0)
        ebb_ps = psb.tile([2, 256], f32, tag="pb")
        nc.tensor.matmul(out=ebb_ps[:, :EMB], lhsT=one2[:], rhs=embb_r[:], start=True, stop=True)
        csum = small.tile([2, EMB], f32)
        nc.vector.tensor_tensor(out=csum[:], in0=embw_t[:], in1=ebb_ps[:2, :EMB], op=AOp.add)
        c_table = const.tile([2, EMB], f32)
        nc.scalar.activation(c_table[:], csum[:], ACT.Tanh)

        W_t, asb, adb, bb, t0c, t1c, dcol = [], [], [], [], [], [], []
        for l in range(3):
            w = const.tile([CI[l], GCN[l]], f32, tag=f"W{l}")
            nc.sync.dma_start(w[:], Ws[l].ap())
            W_t.append(w)
            asb.append(bcast_row(asr[l].ap(), GCN[l]))
            adb.append(bcast_row(adr[l].ap(), GCN[l]))
            bb.append(bcast_row(br[l].ap(), GCN[l]))
            # v = We @ ae  -> tv = c_table . v -> broadcast [128, 2]
            we_t = small.tile([EMB, GCN[l]], f32)
            nc.sync.dma_start(we_t[:], Wer[l].ap())
            aeb = bcast_row(aer[l].ap(), GCN[l])[:EMB, :]
            junk16 = small.tile([EMB, GCN[l]], f32, tag="junk16")
            v16 = small.tile([EMB, 1], f32, tag="v16")
            nc.vector.scalar_tensor_tensor(
                out=junk16[:], in0=we_t[:], scalar=1.0, in1=aeb,
                op0=AOp.mult, op1=AOp.mult, accum_out=v16[:])
            vrow_ps = psb.tile([1, 256], f32, tag="pb")
            nc.tensor.transpose(out=vrow_ps[:1, :EMB], in_=v16[:], identity=ident[:EMB, :EMB])
            vrow = small.tile([1, EMB], f32, tag="vrow_s")
            nc.vector.tensor_copy(vrow[:], vrow_ps[:1, :EMB])
            vb_ps = psb.tile([2, 256], f32, tag="pb")
            nc.tensor.matmul(out=vb_ps[:, :EMB], lhsT=one2[:], rhs=vrow[:], start=True, stop=True)
            junk2 = small.tile([2, EMB], f32, tag="junk2")
            tv = small.tile([2, 1], f32, tag="tv")
            nc.vector.scalar_tensor_tensor(
                out=junk2[:], in0=c_table[:], scalar=1.0, in1=vb_ps[:2, :EMB],
                op0=AOp.mult, op1=AOp.mult, accum_out=tv[:])
            trow_ps = psb.tile([1, 256], f32, tag="pb")
            nc.tensor.transpose(out=trow_ps[:1, :2], in_=tv[:], identity=ident[:2, :2])
            trow = small.tile([1, 2], f32, tag="trow_s")
            nc.vector.tensor_copy(trow[:], trow_ps[:1, :2])
            tc_ps = psb.tile([128, 256], f32, tag="pb")
            nc.tensor.matmul(out=tc_ps[:, :2], lhsT=ones_col[:], rhs=trow[:], start=True, stop=True)
            tcol = const.tile([128, 2], f32, tag=f"tcol{l}")
            nc.vector.tensor_copy(tcol[:], tc_ps[:, :2])
            dc = const.tile([128, 1], f32, tag=f"dc{l}")
            nc.vector.tensor_tensor(out=dc[:], in0=tcol[:, 1:2], in1=tcol[:, 0:1], op=AOp.subtract)
            t0c.append(tcol[:, 0:1]); t1c.append(tcol[:, 1:2]); dcol.append(dc)

        d0wp_t = const.tile([128, 256], f32)
        nc.sync.dma_start(d0wp_t[:], d0wp.ap())
        d1w_t = const.tile([128, 64], f32)
        nc.sync.dma_start(d1w_t[:], d1w.ap())
        uvbias = const.tile([128, 256], f32)
        nc.gpsimd.memset(uvbias[:], 0.0)
        d0bb = bcast_row(d0br.ap(), 128)
        nc.vector.tensor_copy(uvbias[:, 0:128], d0bb[:])
        d1bb = bcast_row(d1br.ap(), 64)
        outwb = bcast_row(outw.ap(), 64)
        outbb = bcast_row(outb.ap(), 1)

        # ---------- L0 prep: xlp0 rows = [x@W0 + b0 | als | ald] ----------
        def prep_rows(xsrc_sbuf, ci, l_next, agrow_slice, is_uv):
            """xsrc_sbuf: [128, ci] node features -> write prep rows to DRAM."""
            xT_ps = ps.tile([128, 128], f32, tag="pT")
            nc.tensor.transpose(out=xT_ps[:ci, :], in_=xsrc_sbuf, identity=ident[:])
            xT = work.tile([128, 128], f32, tag="xTs")
            nc.vector.tensor_copy(xT[:ci, :], xT_ps[:ci, :])
            if is_uv:
                uv_ps = ps.tile([128, 256], f32, tag="pmm")
                nc.tensor.matmul(out=uv_ps[:], lhsT=xT[:ci, :], rhs=d0wp_t[:],
                                 start=True, stop=True)
                uvrow = work.tile([128, 256], f32, tag="uvrow")
                nc.vector.tensor_tensor(out=uvrow[:], in0=uv_ps[:], in1=uvbias[:], op=AOp.add)
                nc.sync.dma_start(agrow_slice, uvrow[:])
                return
            co = GCN[l_next]
            xl_ps = ps.tile([128, 256], f32, tag="pmm")
            nc.tensor.matmul(out=xl_ps[:, :co], lhsT=xT[:ci, :], rhs=W_t[l_next][:],
                             start=True, stop=True)
            agrow = work.tile([128, ROWL[l_next]], f32, tag="agrow")
            nc.gpsimd.memset(agrow[:], 0.0)
            junk = work.tile([128, 128], f32, tag="hTs")
            nc.vector.scalar_tensor_tensor(
                out=junk[:, :co], in0=xl_ps[:, :co], scalar=1.0, in1=asb[l_next][:],
                op0=AOp.mult, op1=AOp.mult, accum_out=agrow[:, co:co + 1])
            nc.vector.scalar_tensor_tensor(
                out=junk[:, :co], in0=xl_ps[:, :co], scalar=1.0, in1=adb[l_next][:],
                op0=AOp.mult, op1=AOp.mult, accum_out=agrow[:, co + 1:co + 2])
            nc.vector.tensor_tensor(out=agrow[:, :co], in0=xl_ps[:, :co],
                                    in1=bb[l_next][:], op=AOp.add)
            nc.sync.dma_start(agrow_slice, agrow[:])

        for i in range(ITERS):
            xt = work.tile([128, 8], f32, tag="x0")
            nc.sync.dma_start(xt[:], x_mine.ap()[i * 128:(i + 1) * 128, :])
            prep_rows(xt[:], 8, 0, ag_in[0].ap()[i * 128:(i + 1) * 128, :], False)
        nc.gpsimd.collective_compute("AllGather", AOp.bypass, replica_groups=RG,
                                     ins=[ag_in[0].ap()], outs=[xlp[0].ap()])

        # ---------- GAT layers ----------
        for l in range(3):
            co = GCN[l]
            R = ROWL[l]
            off = 0
            for i in range(ITERS):
                S = S_list[i]
                xg = big.tile([128, S_list[-1] * ROWL[2]], f32, tag="xg")
                xg3 = xg[:, :S * R].rearrange("p (k c) -> p k c", c=R)
                nc.gpsimd.dma_gather(
                    xg3, xlp[l].ap(), ell_idx_t[:, 8 * off:8 * (off + S)],
                    S * 128, S * 128, R, single_packet=False)
                alpha_t = small.tile([128, S_list[-1]], f32, tag="alpha", name="alpha_t")
                alpha = alpha_t[:, :S]
                nc.vector.scalar_tensor_tensor(
                    out=alpha, in0=ell_cls_t[:, off:off + S], scalar=dcol[l][:],
                    in1=xg3[:, :, co], op0=AOp.mult, op1=AOp.add)
                aldt0 = small.tile([128, 1], f32, tag="aldt0")
                nc.vector.tensor_scalar(out=aldt0[:], in0=xg3[:, 0:1, co + 1],
                                        scalar1=t0c[l], scalar2=None, op0=AOp.add)
                nc.vector.tensor_scalar(out=alpha, in0=alpha, scalar1=aldt0[:],
                                        scalar2=None, op0=AOp.add)
                nc.vector.scalar_tensor_tensor(
                    out=alpha, in0=alpha, scalar=0.2, in1=alpha,
                    op0=AOp.mult, op1=AOp.max)
                pre_t = small.tile([128, S_list[-1]], f32, tag="pre", name="pre_t")
                pre = pre_t[:, :S]
                nc.vector.scalar_tensor_tensor(
                    out=pre, in0=alpha, scalar=100.0, in1=ell_mask_t[:, off:off + S],
                    op0=AOp.add, op1=AOp.mult)
                m100 = small.tile([128, 1], f32, tag="m100")
                nc.vector.tensor_reduce(out=m100[:], in_=pre, axis=mybir.AxisListType.X,
                                        op=AOp.max)
                negb = small.tile([128, 1], f32, tag="negb")
                nc.vector.tensor_scalar(out=negb[:], in0=m100[:], scalar1=-1.0,
                                        scalar2=None, op0=AOp.mult)
                ex_t = small.tile([128, S_list[-1]], f32, tag="ex", name="ex_t")
                ex = ex_t[:, :S]
                ssum = small.tile([128, 1], f32, tag="ssum")
                nc.scalar.activation(ex, pre, ACT.Exp, bias=negb[:], scale=1.0,
                                     accum_out=ssum[:])
                nc.vector.tensor_scalar(out=ssum[:], in0=ssum[:], scalar1=1e-16,
                                        scalar2=None, op0=AOp.add)
                nc.vector.reciprocal(out=ssum[:], in_=ssum[:])
                wexp_t = small.tile([128, S_list[-1]], f32, tag="wexp", name="wexp_t")
                wexp = wexp_t[:, :S]
                nc.vector.tensor_scalar(out=wexp, in0=ex, scalar1=ssum[:],
                                        scalar2=None, op0=AOp.mult)
                nc.vector.tensor_tensor(
                    out=xg3, in0=xg3,
                    in1=wexp.unsqueeze(2).broadcast_to([128, S, R]), op=AOp.mult)
                agg = work.tile([128, 128], f32, tag="agg")
                nc.vector.tensor_reduce(
                    out=agg[:, :co], in_=xg3.transpose([0, 2, 1])[:, 0:co, :],
                    axis=mybir.AxisListType.X, op=AOp.add)
                xnew = work.tile([128, 128], f32, tag="xnew")
                nc.scalar.activation(xnew[:, :co], agg[:, :co], ACT.Tanh)
                if l < 2:
                    prep_rows(xnew[:, :co], co, l + 1,
                              ag_in[l + 1].ap()[i * 128:(i + 1) * 128, :], False)
                else:
                    prep_rows(xnew[:, :co], co, None,
                              ag_in[3].ap()[i * 128:(i + 1) * 128, :], True)
                off += S
            tgt = xlp[l + 1].ap() if l < 2 else uv.ap()
            nc.gpsimd.collective_compute("AllGather", AOp.bypass, replica_groups=RG,
                                         ins=[ag_in[l + 1].ap()], outs=[tgt])

        # ---------- edge MLP over pairs ----------
        valbuf = const.tile([128, NCOL], f32)
        for j in range(NCH):
            uai = small.tile([128, 32], i16, tag="uai")
            nc.sync.dma_start(uai[:], ua_idx.ap()[:, 32 * j:32 * (j + 1)])
            vbi = small.tile([128, 32], i16, tag="vbi")
            nc.sync.dma_start(vbi[:], vb_idx.ap()[:, 32 * j:32 * (j + 1)])
            uat = work.tile([128, 4 * 128], f32, tag="uat")
            ua3 = uat[:].rearrange("p (k c) -> p k c", c=128)
            nc.gpsimd.dma_gather(ua3, uv.ap()[:, 0:128],
                                 uai[:], PAIR_CHUNK, PAIR_CHUNK,
                                 128, elem_step=256)
            vbt = work.tile([128, 4 * 128], f32, tag="vbt")
            vb3 = vbt[:].rearrange("p (k c) -> p k c", c=128)
            nc.gpsimd.dma_gather(vb3, uv.ap()[:, 128:256],
                                 vbi[:], PAIR_CHUNK, PAIR_CHUNK,
                                 128, elem_step=256)
            h0 = work.tile([128, 512], f32, tag="h0")
            nc.vector.tensor_tensor(out=h0[:], in0=uat[:], in1=vbt[:], op=AOp.add)
            nc.scalar.activation(h0[:], h0[:], ACT.Tanh)
            h1_ps = ps.tile([128, 256], f32, tag="pmm")
            for q in range(4):
                hT_ps = ps.tile([128, 128], f32, tag="pT")
                nc.tensor.transpose(out=hT_ps[:], in_=h0[:, 128 * q:128 * (q + 1)],
                                    identity=ident[:])
                hT = work.tile([128, 128], f32, tag="hTs")
                nc.vector.tensor_copy(hT[:], hT_ps[:])
                nc.tensor.matmul(out=h1_ps[:, 64 * q:64 * (q + 1)], lhsT=hT[:],
                                 rhs=d1w_t[:], start=True, stop=True)
            h1 = work.tile([128, 256], f32, tag="h1")
            h13 = h1[:].rearrange("p (k c) -> p k c", c=64)
            nc.vector.tensor_tensor(
                out=h13, in0=h1_ps[:].rearrange("p (k c) -> p k c", c=64),
                in1=d1bb[:].unsqueeze(1).broadcast_to([128, 4, 64]), op=AOp.add)
            nc.scalar.activation(h1[:], h1[:], ACT.Tanh)
            nc.vector.tensor_tensor(
                out=h13, in0=h13,
                in1=outwb[:].unsqueeze(1).broadcast_to([128, 4, 64]), op=AOp.mult)
            nc.vector.tensor_reduce(out=valbuf[:, 4 * j:4 * (j + 1)], in_=h13,
                                    axis=mybir.AxisListType.X, op=AOp.add)

        nc.vector.tensor_scalar(out=valbuf[:], in0=valbuf[:], scalar1=outbb[:, 0:1],
                                scalar2=None, op0=AOp.add)
        exb = const.tile([128, NCOL], f32)
        nc.scalar.activation(exb[:], valbuf[:], ACT.Exp)
        nc.vector.tensor_tensor(out=exb[:], in0=exb[:], in1=pm_t[:], op=AOp.mult)
        ex_table_v = ex_table.ap().rearrange("(j p) c -> p j c", p=128)
        for t in range(NCOL // 28):
            exrep = work.tile([128, 28 * 64], f32, tag="exrep")
            exrep3 = exrep[:].rearrange("p (k c) -> p k c", c=64)
            nc.vector.tensor_copy(
                exrep3, exb[:, 28 * t:28 * (t + 1)].unsqueeze(2).broadcast_to([128, 28, 64]))
            nc.sync.dma_start(ex_table_v[:, 28 * t:28 * (t + 1), :], exrep3)
        for t in range(2):
            sg = small.tile([128, 1], f32, tag="sg")
            for g in range(KG // 32):
                gpi = small.tile([128, 256], i16, tag="gpi")
                nc.sync.dma_start(
                    gpi[:], gph_idx.ap()[:, t * KG * 8 + 256 * g:t * KG * 8 + 256 * (g + 1)])
                gg = work.tile([128, 32 * 64], f32, tag="gg")
                gg3 = gg[:].rearrange("p (k c) -> p k c", c=64)
                nc.gpsimd.dma_gather(gg3, ex_table.ap(), gpi[:], 32 * 128, 32 * 128, 64, single_packet=False)
                gsum = small.tile([128, 1], f32, tag="gsum")
                nc.vector.tensor_reduce(out=gsum[:], in_=gg3[:, :, 0:1].squeeze(2),
                                        axis=mybir.AxisListType.X, op=AOp.add)
                if g == 0:
                    nc.vector.tensor_copy(sg[:], gsum[:])
                else:
                    nc.vector.tensor_tensor(out=sg[:], in0=sg[:], in1=gsum[:], op=AOp.add)
            nc.sync.dma_start(ar_in.ap()[t * 128:(t + 1) * 128, :], sg[:])
        nc.gpsimd.collective_compute("AllReduce", AOp.add, replica_groups=RG,
                                     ins=[ar_in.ap()], outs=[ar_out.ap()])
        for t in range(2):
            st = small.tile([128, 1], f32, tag="st")
            nc.sync.dma_start(st[:], ar_out.ap()[t * 128:(t + 1) * 128, :])
            nc.vector.tensor_scalar(out=st[:], in0=st[:], scalar1=1e-16,
                                    scalar2=None, op0=AOp.add)
            nc.vector.reciprocal(out=st[:], in_=st[:])
            srep = work.tile([128, 64], f32, tag="srep")
            nc.vector.tensor_copy(srep[:], st[:].broadcast_to([128, 64]))
            nc.sync.dma_start(s_table.ap()[t * 128:(t + 1) * 128, :], srep[:])
        smt = const.tile([128, NCOL], f32)
        for t in range(NCOL // 28):
            ebi = small.tile([128, 224], i16, tag="ebi")
            nc.sync.dma_start(ebi[:], eb_idx.ap()[:, 224 * t:224 * (t + 1)])
            sgt = work.tile([128, 28 * 64], f32, tag="sgt")
            sgt3 = sgt[:].rearrange("p (k c) -> p k c", c=64)
            nc.gpsimd.dma_gather(sgt3, s_table.ap(), ebi[:], 28 * 128, 28 * 128, 64, single_packet=False)
            nc.vector.tensor_tensor(out=smt[:, 28 * t:28 * (t + 1)],
                                    in0=exb[:, 28 * t:28 * (t + 1)],
                                    in1=sgt3[:, :, 0:1].squeeze(2), op=AOp.mult)
        nc.sync.dma_start(sm_out.ap().rearrange("(j p) -> p j", p=128), smt[:])

    nc.compile()
    return nc


_CACHE = {}


def kernel(**inputs):
    inputs = {k: np.asarray(v) for k, v in inputs.items()}
    x = inputs["x"].astype(F32)
    edges = inputs["edges"]
    core_in, meta, e_out, ec_host = _preprocess(
        x, edges, inputs["edge_attr"].astype(F32),
        np.asarray(inputs["detector_labels"], bool), inputs["batch_labels"])

    params = {}
    params["emb_W"] = inputs["emb_W"].astype(F32)
    params["emb_b"] = inputs["emb_b"].astype(F32)[None, :]
    for l in range(3):
        W = inputs[f"gat{l}_W"].astype(F32)
        Wp = np.zeros((CI[l], GCN[l]), F32)
        Wp[:W.shape[0]] = W
        params[f"W{l}"] = Wp
        params[f"as{l}"] = inputs[f"gat{l}_as"].astype(F32)[None, :]
        params[f"ad{l}"] = inputs[f"gat{l}_ad"].astype(F32)[None, :]
        params[f"We{l}"] = inputs[f"gat{l}_We"].astype(F32)
        params[f"ae{l}"] = inputs[f"gat{l}_ae"].astype(F32)[None, :]
        params[f"b{l}"] = inputs[f"gat{l}_b"].astype(F32)[None, :]
    d0W = inputs["d0_W"].astype(F32)
    params["d0wp"] = np.concatenate([d0W[:128, :], d0W[128:, :]], axis=1)
    params["d0br"] = inputs["d0_b"].astype(F32)[None, :]
    params["d1w"] = inputs["d1_W"].astype(F32)
    params["d1br"] = inputs["d1_b"].astype(F32)[None, :]
    params["outw"] = inputs["out_W"].astype(F32).reshape(1, 64)
    params["outb"] = inputs["out_b"].astype(F32).reshape(1, 1)

    key = tuple(meta["S_list"]) + (meta["PPC"], meta["KG"])
    if key not in _CACHE:
        _CACHE[key] = _build_program(meta, params)
    nc = _CACHE[key]

    in_maps = []
    for c in range(NC):
        m = dict(params)
        m.update(core_in[c])
        in_maps.append(m)

    from concourse import bass2jax
    results = bass2jax.run_bass_via_pjrt(nc, in_maps, n_cores=NC)

    ppc = meta["ppc"]
    sm = np.concatenate([results[c]["sm_out"][:ppc] for c in range(NC)])
    return (e_out, sm.astype(F32), ec_host)
